# revision 22
# baseline (speedup 1.0000x reference)
"""GAT layer (nn_GATLayer) on 8 Trainium2 NeuronCores.

Row-parallel sharding: core c owns rows [c*1024, (c+1)*1024) of the
attention matrix. Each core receives `doc` rolled by -c*1024 rows so a
single static kernel (rows 0..1023 of its local order) serves all cores;
the host un-rolls the att columns when gathering.

Per-core pipeline:
  prologue: hT = W @ docT (f32r matmuls), s_src/s_dst scores, s_dst
            broadcast (K=1 ones matmul), per-chunk bias columns,
            h in [j, d] bf16 layout via PE transposes.
  main (8 chunks of 128 rows x 8192 cols):
    ACT: L = Prelu(s_dst_rep + s_src)         (leaky-relu scores)
    ACT: E = Exp(L) -> bf16, accum_out = rowsum(E)
    DVE: zero diagonal block (mask mult), s = rowsum - E_diag, 1/s
    DVE: att = E * (1/s) -> fp32 -> DMA out
    PE : E_T tiles via transpose (bf16) ; h_out = E_T.T @ h (psum accum)
    ACT: h_out = Prelu(psum * 1/s) -> DMA out
"""

import sys

if "/opt/trn_rl_repo" not in sys.path:
    sys.path.insert(0, "/opt/trn_rl_repo")

import numpy as np

import concourse.bass as bass
import concourse.tile as tile
from concourse import bacc, mybir
from concourse.bass import ds
from concourse.bass_utils import run_bass_kernel_spmd
from concourse.masks import make_identity

N = 8192
IN_FEAT = 256
D = 128
NCORES = 8
R = N // NCORES          # 1024 rows per core
NCHUNK = R // 128        # 8 chunks of 128 rows
SLOPE = 0.1
HALF = N // 2            # 4096

F32 = mybir.dt.float32
F32R = mybir.dt.float32r
BF16 = mybir.dt.bfloat16
AF = mybir.ActivationFunctionType
ALU = mybir.AluOpType

LAST_RESULTS = None      # test.py reads exec_time_ns off this


def _build(a_b: float, self_link: int) -> bass.Bass:
    nc = bacc.Bacc("TRN2", target_bir_lowering=False)

    docT = nc.dram_tensor("docT", (IN_FEAT, N), F32R, kind="ExternalInput")
    WT = nc.dram_tensor("WT", (IN_FEAT, D), F32R, kind="ExternalInput")
    Wb = nc.dram_tensor("Wb", (D, 1), F32, kind="ExternalInput")
    a_cat = nc.dram_tensor("a_cat", (D, 2), F32R, kind="ExternalInput")
    att_blk = nc.dram_tensor("att_blk", (R, N), F32, kind="ExternalOutput")
    hout_blk = nc.dram_tensor("hout_blk", (R, D), F32, kind="ExternalOutput")

    with tile.TileContext(nc) as tc:
        const = tc.alloc_tile_pool(name="const", bufs=1)
        small = tc.alloc_tile_pool(name="small", bufs=3)
        pro = tc.alloc_tile_pool(name="pro", bufs=1)
        ppro = tc.alloc_tile_pool(name="ppro", bufs=2, space="PSUM")

        # ---------------- constants / parameters ----------------
        I128b = const.tile([128, 128], BF16, name="I128b")
        make_identity(nc, I128b)
        dmask = const.tile([128, 128], BF16, name="dmask")
        if self_link == 0:
            nc.gpsimd.memset(dmask, 1.0)
            nc.gpsimd.affine_select(
                out=dmask,
                in_=dmask,
                compare_op=ALU.not_equal,
                fill=0.0,
                base=0,
                pattern=[[-1, 128]],
                channel_multiplier=1,
            )

        wt = pro.tile([128, 2, 128], F32R, name="wt")
        nc.sync.dma_start(wt, WT[:].rearrange("(k p) d -> p k d", p=128))
        wb = const.tile([128, 1], F32, name="wb")
        nc.sync.dma_start(wb, Wb[:])
        acat = pro.tile([128, 2], F32R, name="acat")
        nc.sync.dma_start(acat, a_cat[:])

        ones1 = pro.tile([1, 128], F32, name="ones1")
        nc.vector.memset(ones1, 1.0)
        ones11 = pro.tile([1, 1], F32, name="ones11")
        nc.vector.memset(ones11, 1.0)
        ab_col = const.tile([128, 1], F32, name="ab_col")
        nc.vector.memset(ab_col, float(a_b))

        # ---------------- hT = (W @ doc.T) + Wb : [d=128, i=8192] ----
        # doc.T streamed in column halves to bound SBUF. hT kept twice:
        # f32r for the score matmuls, bf16 for the h-layout transposes.
        hT = pro.tile([128, N], F32R, name="hT")
        hT_bf = pro.tile([128, N], BF16, name="hT_bf")
        for h in range(2):
            dT = pro.tile([128, 2, HALF], F32R, tag="dT")
            nc.sync.dma_start(
                dT,
                docT[:, ds(h * HALF, HALF)].rearrange("(k p) n -> p k n", p=128),
            )
            for w in range(8):
                ph = ppro.tile([128, 512], F32, tag="pp")
                for k in range(2):
                    nc.tensor.matmul(
                        ph,
                        wt[:, k, :],
                        dT[:, k, ds(w * 512, 512)],
                        start=(k == 0),
                        stop=(k == 1),
                    )
                # copy + per-partition bias on ACT
                nc.scalar.activation(
                    hT[:, ds(h * HALF + w * 512, 512)], ph, AF.Identity, bias=wb
                )
                nc.scalar.activation(
                    hT_bf[:, ds(h * HALF + w * 512, 512)], ph, AF.Identity, bias=wb
                )

        # ---------------- scores s_src/s_dst : 2x [1, 8192] ----------
        s_src_row = pro.tile([1, N], F32, name="s_src_row")
        s_dst_row = pro.tile([1, N], F32, name="s_dst_row")
        for w in range(16):
            for idx, dst in ((0, s_src_row), (1, s_dst_row)):
                psx = ppro.tile([1, 512], F32, tag="pp")
                nc.tensor.matmul(
                    psx,
                    acat[:, idx : idx + 1],
                    hT[:, ds(w * 512, 512)],
                )
                nc.vector.tensor_copy(dst[:, ds(w * 512, 512)], psx)

        # ---------------- s_dst broadcast + a_b : [128, 8192] --------
        sdst_rep = [
            const.tile([128, HALF], F32, name=f"sdst{h}") for h in range(2)
        ]
        for w in range(16):
            pr = ppro.tile([128, 512], F32, tag="pp")
            nc.tensor.matmul(
                pr,
                ones1,
                s_dst_row[:, ds(w * 512, 512)],
            )
            nc.scalar.activation(
                sdst_rep[w // 8][:, ds((w % 8) * 512, 512)],
                pr,
                AF.Identity,
                bias=ab_col,
            )

        # ------------- per-chunk bias columns [128, 8] ----------------
        scol_src = const.tile([128, NCHUNK], F32, name="scol_src")
        scol_dst = const.tile([128, NCHUNK], F32, name="scol_dst")
        for c in range(NCHUNK):
            for src_row, dst in ((s_src_row, scol_src), (s_dst_row, scol_dst)):
                pc = ppro.tile([128, 1], F32, tag="pp")
                nc.tensor.matmul(
                    pc,
                    src_row[:, ds(c * 128, 128)],
                    ones11,
                )
                nc.vector.tensor_copy(dst[:, c : c + 1], pc)
        # diag score = s_src[i] + s_dst[i] + a_b; E_ii activation reads
        # scol_dst as in_, so its bias carries s_src + a_b.
        bias2 = const.tile([128, NCHUNK], F32, name="bias2")
        nc.vector.tensor_scalar_add(bias2, scol_src, float(a_b))

        # ------------- h in [j, d] bf16 layout ------------------------
        h_bf = const.tile([128, 64, 128], BF16, name="h_bf")
        for w in range(16):
            phb = ppro.tile([128, 4, 128], BF16, tag="ppb")
            for q in range(4):
                nc.tensor.transpose(
                    phb[:, q], hT_bf[:, ds((4 * w + q) * 128, 128)], I128b
                )
            nc.vector.tensor_copy(h_bf[:, ds(4 * w, 4), :], phb)

        ppro.release()
        pro.release()

        work = tc.alloc_tile_pool(name="work", bufs=2)
        attp = tc.alloc_tile_pool(name="attp", bufs=3)
        ptp = tc.alloc_tile_pool(name="ptp", bufs=3, space="PSUM")
        pho = tc.alloc_tile_pool(name="pho", bufs=2, space="PSUM")

        # =================== main loop over 8 row chunks ==============
        for c in range(NCHUNK):
            sacc = small.tile([128, 2], F32, tag="sacc")
            nc.vector.memset(sacc, 0.0)
            E = work.tile([128, N], BF16, tag="E")
            for h in range(2):
                L = work.tile([128, HALF], F32, tag="L")
                nc.scalar.activation(
                    L,
                    sdst_rep[h],
                    AF.Prelu,
                    bias=scol_src[:, c : c + 1],
                    alpha=SLOPE,
                )
                nc.scalar.activation(
                    E[:, ds(h * HALF, HALF)],
                    L,
                    AF.Exp,
                    accum_out=sacc[:, h : h + 1],
                )

            s_c = small.tile([128, 1], F32, tag="s_c")
            nc.vector.tensor_tensor(s_c, sacc[:, 0:1], sacc[:, 1:2], ALU.add)
            if self_link == 0:
                # zero the diagonal block; subtract its exp from rowsum
                nc.vector.tensor_tensor(
                    E[:, ds(c * 128, 128)],
                    E[:, ds(c * 128, 128)],
                    dmask,
                    ALU.mult,
                )
                t1 = small.tile([128, 1], F32, tag="t1")
                nc.scalar.activation(
                    t1,
                    scol_dst[:, c : c + 1],
                    AF.Prelu,
                    bias=bias2[:, c : c + 1],
                    alpha=SLOPE,
                )
                t2 = small.tile([128, 1], F32, tag="t2")
                nc.scalar.activation(t2, t1, AF.Exp)
                nc.vector.tensor_tensor(s_c, s_c, t2, ALU.subtract)

            sinv = small.tile([128, 1], F32, tag="sinv")
            nc.vector.reciprocal(sinv, s_c)

            # att rows out
            for q in range(4):
                at = attp.tile([128, 2048], F32, tag="at")
                nc.vector.tensor_scalar_mul(at, E[:, ds(q * 2048, 2048)], sinv)
                nc.sync.dma_start(
                    att_blk[ds(c * 128, 128), ds(q * 2048, 2048)], at
                )

            # E_T tiles + matmul
            ET = work.tile([128, 64, 128], BF16, tag="ET")
            for t in range(16):
                pt = ptp.tile([128, 4, 128], BF16, tag="pt")
                for q in range(4):
                    nc.tensor.transpose(
                        pt[:, q], E[:, ds((4 * t + q) * 128, 128)], I128b
                    )
                nc.vector.tensor_copy(ET[:, ds(4 * t, 4), :], pt)

            ph_o = pho.tile([128, 128], F32, tag="ph_o")
            for jt in range(64):
                nc.tensor.matmul(
                    ph_o,
                    ET[:, jt, :],
                    h_bf[:, jt, :],
                    start=(jt == 0),
                    stop=(jt == 63),
                )
            ho = small.tile([128, 128], F32, tag="ho")
            nc.scalar.activation(ho, ph_o, AF.Prelu, scale=sinv, alpha=SLOPE)
            nc.sync.dma_start(hout_blk[ds(c * 128, 128), :], ho)

        pho.release()
        ptp.release()
        attp.release()
        work.release()
        small.release()
        const.release()

    nc.compile()
    return nc


def kernel(doc, W_w, W_b, a_w, a_b, selfLink):
    global LAST_RESULTS
    doc = np.ascontiguousarray(np.asarray(doc, dtype=np.float32))
    W_w = np.asarray(W_w, dtype=np.float32)
    W_b = np.asarray(W_b, dtype=np.float32)
    a_w = np.asarray(a_w, dtype=np.float32)
    a_b_f = float(np.asarray(a_b, dtype=np.float32).reshape(-1)[0])
    sl = np.asarray(selfLink)
    self_link = int(sl.reshape(-1)[0]) if sl.size else 0

    nc = _build(a_b_f, self_link)

    WT = np.ascontiguousarray(W_w.T)                      # [256, 128]
    Wbv = np.ascontiguousarray(W_b.reshape(D, 1))         # [128, 1]
    acat = np.ascontiguousarray(
        np.stack([a_w[0, :D], a_w[0, D:]], axis=1)        # [128, 2]
    )

    in_maps = []
    for c in range(NCORES):
        docR = np.roll(doc, -c * R, axis=0)
        in_maps.append(
            {
                "docT": np.ascontiguousarray(docR.T),
                "WT": WT,
                "Wb": Wbv,
                "a_cat": acat,
            }
        )

    LAST_RESULTS = run_bass_kernel_spmd(
        nc, in_maps, core_ids=list(range(NCORES))
    )
    results = LAST_RESULTS.results

    att = np.empty((N, N), dtype=np.float32)
    hout = np.empty((N, D), dtype=np.float32)
    for c in range(NCORES):
        att[c * R : (c + 1) * R] = np.roll(results[c]["att_blk"], c * R, axis=1)
        hout[c * R : (c + 1) * R] = results[c]["hout_blk"]
    return (hout, att)


# revision 30
# speedup vs baseline: 749.9616x; 749.9616x over previous
"""GAT layer (nn_GATLayer) on 8 Trainium2 NeuronCores.

Row-parallel sharding: core c owns rows [c*1024, (c+1)*1024) of the
attention matrix. Each core receives `doc` rolled by -c*1024 rows so a
single static kernel (rows 0..1023 of its local order) serves all cores;
the host un-rolls the att columns when gathering.

Per-core pipeline:
  prologue (pipelined in 2048-col groups):
    hT = W @ docT (f32r matmuls, bf16 shadow copy),
    sdst_rep = (ones x a_dst^T) @ hT + a_b  -- one fused matmul per window,
    s_src row (local 1024 only), per-chunk bias columns,
    h in [j, d] bf16 layout via PE transposes.
  main (8 chunks of 128 rows x 8192 cols):
    ACT: L = Prelu(sdst_rep + s_src)          (leaky-relu scores)
    ACT: E = Exp(L) -> bf16, accum_out = rowsum(E)
    DVE: zero diagonal block (mask mult), s = rowsum - E_diag, 1/s
    GPS: att = E * (1/s) -> fp32 -> DMA out   (GpSimd tensor_scalar)
    PE : E_T tiles via transpose (bf16) ; h_out = E_T.T @ h (psum accum)
    ACT: h_out = Prelu(psum * 1/s) -> DMA out
"""

import sys

if "/opt/trn_rl_repo" not in sys.path:
    sys.path.insert(0, "/opt/trn_rl_repo")

import numpy as np

import concourse.bass as bass
import concourse.tile as tile
from concourse import bacc, mybir
from concourse.bass import ds
from concourse.bass_utils import run_bass_kernel_spmd
from concourse.masks import make_identity

N = 8192
IN_FEAT = 256
D = 128
NCORES = 8
R = N // NCORES          # 1024 rows per core
NCHUNK = R // 128        # 8 chunks of 128 rows
SLOPE = 0.1
HALF = N // 2            # 4096
GRP = 2048               # prologue pipeline granularity (4 groups)

F32 = mybir.dt.float32
F32R = mybir.dt.float32r
BF16 = mybir.dt.bfloat16
F16 = mybir.dt.float16
AF = mybir.ActivationFunctionType
ALU = mybir.AluOpType

# att normalization engine: "gpsimd" or "vector"
ATT_ENGINE = "vector"
# halves (of 16) whose leaky-relu runs on DVE instead of ACT
N_DVE_HALVES = 6

LAST_RESULTS = None      # test.py reads exec_time_ns off this


def _build(a_b: float, self_link: int, reps: int = 1) -> bass.Bass:
    """reps>1 repeats the main loop (same outputs) -- timing use only."""
    nc = bacc.Bacc("TRN2", target_bir_lowering=False)

    docT = nc.dram_tensor("docT", (IN_FEAT, N), F32R, kind="ExternalInput")
    WT = nc.dram_tensor("WT", (IN_FEAT, D), F32R, kind="ExternalInput")
    Wb = nc.dram_tensor("Wb", (D, 1), F32, kind="ExternalInput")
    a_cat = nc.dram_tensor("a_cat", (D, 2), F32, kind="ExternalInput")
    att_blk = nc.dram_tensor("att_blk", (R, N), F32, kind="ExternalOutput")
    hout_blk = nc.dram_tensor("hout_blk", (R, D), F32, kind="ExternalOutput")

    with tile.TileContext(nc) as tc:
        const = tc.alloc_tile_pool(name="const", bufs=1)
        small = tc.alloc_tile_pool(name="small", bufs=3)
        pro = tc.alloc_tile_pool(name="pro", bufs=1)
        ppro = tc.alloc_tile_pool(name="ppro", bufs=3, space="PSUM")

        # ---------------- constants / parameters ----------------
        I128b = const.tile([128, 128], BF16, name="I128b")
        make_identity(nc, I128b)
        I128h = pro.tile([128, 128], F16, name="I128h")
        make_identity(nc, I128h)
        dmask = const.tile([128, 128], BF16, name="dmask")
        if self_link == 0:
            nc.gpsimd.memset(dmask, 1.0)
            nc.gpsimd.affine_select(
                out=dmask,
                in_=dmask,
                compare_op=ALU.not_equal,
                fill=0.0,
                base=0,
                pattern=[[-1, 128]],
                channel_multiplier=1,
            )

        wt = pro.tile([128, 2, 128], F32R, name="wt")
        nc.sync.dma_start(wt, WT[:].rearrange("(k p) d -> p k d", p=128))
        wb = const.tile([128, 1], F32, name="wb")
        nc.sync.dma_start(wb, Wb[:])
        acat_f = pro.tile([128, 2], F32, name="acat_f")
        nc.sync.dma_start(acat_f, a_cat[:])

        # a_src column (f32r) for the s_src matmuls
        asrc_r = pro.tile([128, 1], F32R, name="asrc_r")
        nc.vector.tensor_copy(asrc_r, acat_f[:, 0:1])
        # a_dst replicated across 128 columns (weights for the fused
        # s_dst-broadcast matmul): lhsT[d, m] = a_dst[d] for all m.
        adst_rep = pro.tile([128, 128], F32R, name="adst_rep")
        nc.vector.tensor_copy(adst_rep, acat_f[:, 1:2].to_broadcast((128, 128)))

        ones11 = pro.tile([1, 1], F32, name="ones11")
        nc.vector.memset(ones11, 1.0)
        ab_col = const.tile([128, 1], F32, name="ab_col")
        nc.vector.memset(ab_col, float(a_b))

        # ---------------- hT groups + fused sdst_rep ------------------
        # Per 2048-col group: DMA doc.T quarter, matmul hT, ACT-copy to
        # f32r + bf16, fused broadcast matmul -> sdst_rep (+a_b).
        hT_g = [pro.tile([128, GRP], F32R, name=f"hT{g}") for g in range(4)]
        hTb_g = [pro.tile([128, GRP], BF16, name=f"hTb{g}") for g in range(4)]
        sdst_rep = [
            const.tile([128, HALF], F16, name=f"sdst{h}") for h in range(2)
        ]
        s_src_row = pro.tile([1, R], F32, name="s_src_row")

        for g in range(4):
            dT = pro.tile([128, 2, GRP], F32R, tag="dT", bufs=2)
            nc.sync.dma_start(
                dT,
                docT[:, ds(g * GRP, GRP)].rearrange("(k p) n -> p k n", p=128),
            )
            for w in range(4):
                off = g * GRP + w * 512
                ph = ppro.tile([128, 512], F32, tag="pp")
                for k in range(2):
                    nc.tensor.matmul(
                        ph,
                        wt[:, k, :],
                        dT[:, k, ds(w * 512, 512)],
                        start=(k == 0),
                        stop=(k == 1),
                    )
                nc.scalar.activation(
                    hT_g[g][:, ds(w * 512, 512)], ph, AF.Identity, bias=wb
                )
                nc.scalar.activation(
                    hTb_g[g][:, ds(w * 512, 512)], ph, AF.Identity, bias=wb
                )
                # fused s_dst broadcast: [128,512] = adst_rep.T @ hT window
                pr = ppro.tile([128, 512], F32, tag="pp")
                nc.tensor.matmul(
                    pr, adst_rep, hT_g[g][:, ds(w * 512, 512)]
                )
                nc.vector.tensor_scalar_add(
                    sdst_rep[off // HALF][:, ds(off % HALF, 512)],
                    pr,
                    float(a_b),
                )
                # s_src for local rows (first 1024 cols only)
                if off < R:
                    psx = ppro.tile([1, 512], F32, tag="pp")
                    nc.tensor.matmul(
                        psx, asrc_r, hT_g[g][:, ds(w * 512, 512)]
                    )
                    nc.vector.tensor_copy(s_src_row[:, ds(off, 512)], psx)

        # ------------- per-chunk bias columns [128, 8] ----------------
        # scol_dst_ab[:, c] = s_dst[c*128 + p] + a_b  (transpose of the
        # replicated sdst_rep block); scol_src[:, c] = s_src[c*128 + p].
        scol_src = const.tile([128, NCHUNK], F32, name="scol_src")
        scol_dst_ab = const.tile([128, NCHUNK], F32, name="scol_dst_ab")
        for c in range(NCHUNK):
            pc = ppro.tile([128, 1], F32, tag="pp")
            nc.tensor.matmul(
                pc,
                s_src_row[:, ds(c * 128, 128)],
                ones11,
            )
            nc.vector.tensor_copy(scol_src[:, c : c + 1], pc)
            pt = ppro.tile([128, 128], F16, tag="ppb")
            nc.tensor.transpose(
                pt, sdst_rep[0][:, ds(c * 128, 128)], I128h
            )
            nc.vector.tensor_copy(scol_dst_ab[:, c : c + 1], pt[:, 0:1])

        scol_src01 = const.tile([128, NCHUNK], F32, name="scol_src01")
        nc.vector.tensor_scalar_mul(scol_src01, scol_src, SLOPE)

        # E_ii for all chunks at once: exp(lrelu(s_src + s_dst + a_b))
        eii = const.tile([128, NCHUNK], F32, name="eii")
        if self_link == 0:
            nc.vector.tensor_tensor(eii, scol_dst_ab, scol_src, ALU.add)
            nc.scalar.activation(eii, eii, AF.Prelu, alpha=SLOPE)
            nc.scalar.activation(eii, eii, AF.Exp)

        # ------------- h in [j, d] bf16 layout ------------------------
        h_bf = const.tile([128, 64, 128], BF16, name="h_bf")
        for w in range(8):
            phb = ppro.tile([128, 8, 128], BF16, tag="ppb")
            for q in range(8):
                t_ = 8 * w + q
                g, r_ = divmod(t_ * 128, GRP)
                nc.tensor.transpose(
                    phb[:, q], hTb_g[g][:, ds(r_, 128)], I128b
                )
            nc.vector.tensor_copy(h_bf[:, ds(8 * w, 8), :], phb)

        ppro.release()
        pro.release()

        work = tc.alloc_tile_pool(name="work", bufs=2)
        attp = tc.alloc_tile_pool(name="attp", bufs=4)
        ptp = tc.alloc_tile_pool(name="ptp", bufs=3, space="PSUM")
        pho = tc.alloc_tile_pool(name="pho", bufs=2, space="PSUM")

        att_eng = nc.gpsimd if ATT_ENGINE == "gpsimd" else nc.vector

        # =================== main loop over 8 row chunks ==============
        for c in [c for _ in range(reps) for c in range(NCHUNK)]:
            sacc = small.tile([128, 2], F32, tag="sacc")
            nc.vector.memset(sacc, 0.0)
            E = work.tile([128, N], BF16, tag="E")
            for h in range(2):
                L = work.tile([128, HALF], F16, tag="L")
                if 2 * c + h < N_DVE_HALVES:
                    # leaky-relu on DVE in fp16: max(x, 0.1x)
                    sc = work.tile([128, HALF], F16, tag="sc")
                    nc.vector.tensor_scalar_add(
                        sc, sdst_rep[h], scol_src[:, c : c + 1]
                    )
                    sc01 = work.tile([128, HALF], F16, tag="sc01")
                    nc.vector.tensor_scalar(
                        sc01,
                        sdst_rep[h],
                        SLOPE,
                        scol_src01[:, c : c + 1],
                        ALU.mult,
                        ALU.add,
                    )
                    nc.vector.tensor_tensor(L, sc, sc01, ALU.max)
                else:
                    nc.scalar.activation(
                        L,
                        sdst_rep[h],
                        AF.Prelu,
                        bias=scol_src[:, c : c + 1],
                        alpha=SLOPE,
                    )
                nc.scalar.activation(
                    E[:, ds(h * HALF, HALF)],
                    L,
                    AF.Exp,
                    accum_out=sacc[:, h : h + 1],
                )

            s_c = small.tile([128, 1], F32, tag="s_c")
            nc.vector.tensor_tensor(s_c, sacc[:, 0:1], sacc[:, 1:2], ALU.add)
            if self_link == 0:
                # zero the diagonal block; subtract its exp from rowsum
                nc.vector.tensor_tensor(
                    E[:, ds(c * 128, 128)],
                    E[:, ds(c * 128, 128)],
                    dmask,
                    ALU.mult,
                )
                nc.vector.tensor_tensor(
                    s_c, s_c, eii[:, c : c + 1], ALU.subtract
                )

            sinv = small.tile([128, 1], F32, tag="sinv")
            nc.vector.reciprocal(sinv, s_c)

            # att rows out
            for q in range(4):
                at = attp.tile([128, 2048], F32, tag="at")
                att_eng.tensor_scalar_mul(at, E[:, ds(q * 2048, 2048)], sinv)
                nc.sync.dma_start(
                    att_blk[ds(c * 128, 128), ds(q * 2048, 2048)], at
                )

            # E_T tiles + matmul
            ET = work.tile([128, 64, 128], BF16, tag="ET")
            for t in range(4):
                pt = ptp.tile([128, 16, 128], BF16, tag="pt")
                for q in range(16):
                    nc.tensor.transpose(
                        pt[:, q], E[:, ds((16 * t + q) * 128, 128)], I128b
                    )
                nc.vector.tensor_copy(ET[:, ds(16 * t, 16), :], pt)

            ph_o = pho.tile([128, 128], F32, tag="ph_o")
            for jt in range(64):
                nc.tensor.matmul(
                    ph_o,
                    ET[:, jt, :],
                    h_bf[:, jt, :],
                    start=(jt == 0),
                    stop=(jt == 63),
                )
            ho = small.tile([128, 128], F32, tag="ho")
            nc.scalar.activation(ho, ph_o, AF.Prelu, scale=sinv, alpha=SLOPE)
            nc.sync.dma_start(hout_blk[ds(c * 128, 128), :], ho)

        pho.release()
        ptp.release()
        attp.release()
        work.release()
        small.release()
        const.release()

    nc.compile()
    return nc


def kernel(doc, W_w, W_b, a_w, a_b, selfLink):
    global LAST_RESULTS
    doc = np.ascontiguousarray(np.asarray(doc, dtype=np.float32))
    W_w = np.asarray(W_w, dtype=np.float32)
    W_b = np.asarray(W_b, dtype=np.float32)
    a_w = np.asarray(a_w, dtype=np.float32)
    a_b_f = float(np.asarray(a_b, dtype=np.float32).reshape(-1)[0])
    sl = np.asarray(selfLink)
    self_link = int(sl.reshape(-1)[0]) if sl.size else 0

    nc = _build(a_b_f, self_link)

    WT = np.ascontiguousarray(W_w.T)                      # [256, 128]
    Wbv = np.ascontiguousarray(W_b.reshape(D, 1))         # [128, 1]
    acat = np.ascontiguousarray(
        np.stack([a_w[0, :D], a_w[0, D:]], axis=1)        # [128, 2]
    )

    in_maps = []
    for c in range(NCORES):
        docR = np.roll(doc, -c * R, axis=0)
        in_maps.append(
            {
                "docT": np.ascontiguousarray(docR.T),
                "WT": WT,
                "Wb": Wbv,
                "a_cat": acat,
            }
        )

    LAST_RESULTS = run_bass_kernel_spmd(
        nc, in_maps, core_ids=list(range(NCORES))
    )
    results = LAST_RESULTS.results

    att = np.empty((N, N), dtype=np.float32)
    hout = np.empty((N, D), dtype=np.float32)
    for c in range(NCORES):
        att[c * R : (c + 1) * R] = np.roll(results[c]["att_blk"], c * R, axis=1)
        hout[c * R : (c + 1) * R] = results[c]["hout_blk"]
    return (hout, att)


# revision 35
# speedup vs baseline: 808.0735x; 1.0775x over previous
"""GAT layer (nn_GATLayer) on 8 Trainium2 NeuronCores.

Row-parallel sharding: core c owns rows [c*1024, (c+1)*1024) of the
attention matrix. Each core receives `doc` rolled by -c*1024 rows so a
single static kernel (rows 0..1023 of its local order) serves all cores;
the host un-rolls the att columns when gathering.

Per-core pipeline:
  prologue (pipelined in 2048-col groups):
    hT = W @ docT (f32r matmuls, bf16 shadow copy),
    sdst_rep = (ones x a_dst^T) @ hT + a_b  -- one fused matmul per window,
    s_src row (local 1024 only), per-chunk bias columns,
    h in [j, d] bf16 layout via PE transposes.
  main (8 chunks of 128 rows x 8192 cols):
    ACT: L = Prelu(sdst_rep + s_src)          (leaky-relu scores)
    ACT: E = Exp(L) -> bf16, accum_out = rowsum(E)
    DVE: zero diagonal block (mask mult), s = rowsum - E_diag, 1/s
    GPS: att = E * (1/s) -> fp32 -> DMA out   (GpSimd tensor_scalar)
    PE : E_T tiles via transpose (bf16) ; h_out = E_T.T @ h (psum accum)
    ACT: h_out = Prelu(psum * 1/s) -> DMA out
"""

import sys

if "/opt/trn_rl_repo" not in sys.path:
    sys.path.insert(0, "/opt/trn_rl_repo")

import numpy as np

import concourse.bass as bass
import concourse.tile as tile
from concourse import bacc, mybir
from concourse.bass import ds
from concourse.bass_utils import run_bass_kernel_spmd
from concourse.masks import make_identity

N = 8192
IN_FEAT = 256
D = 128
NCORES = 8
R = N // NCORES          # 1024 rows per core
NCHUNK = R // 128        # 8 chunks of 128 rows
SLOPE = 0.1
HALF = N // 2            # 4096
GRP = 2048               # prologue pipeline granularity (4 groups)

F32 = mybir.dt.float32
F32R = mybir.dt.float32r
BF16 = mybir.dt.bfloat16
F16 = mybir.dt.float16
AF = mybir.ActivationFunctionType
ALU = mybir.AluOpType

# att normalization engine: "gpsimd" or "vector"
ATT_ENGINE = "vector"
# halves (of 16) whose leaky-relu runs on DVE instead of ACT
N_DVE_HALVES = 6

LAST_RESULTS = None      # test.py reads exec_time_ns off this


def _build(a_b: float, self_link: int, reps: int = 1) -> bass.Bass:
    """reps>1 repeats the main loop (same outputs) -- timing use only."""
    nc = bacc.Bacc("TRN2", target_bir_lowering=False)

    docT = nc.dram_tensor("docT", (IN_FEAT, N), F32R, kind="ExternalInput")
    WT = nc.dram_tensor("WT", (IN_FEAT, D), F32R, kind="ExternalInput")
    Wb = nc.dram_tensor("Wb", (D, 1), F32, kind="ExternalInput")
    a_cat = nc.dram_tensor("a_cat", (D, 2), F32, kind="ExternalInput")
    att_blk = nc.dram_tensor("att_blk", (R, N), F32, kind="ExternalOutput")
    hout_blk = nc.dram_tensor("hout_blk", (R, D), F32, kind="ExternalOutput")

    with tile.TileContext(nc) as tc:
        const = tc.alloc_tile_pool(name="const", bufs=1)
        small = tc.alloc_tile_pool(name="small", bufs=3)
        pro = tc.alloc_tile_pool(name="pro", bufs=1)
        ppro = tc.alloc_tile_pool(name="ppro", bufs=3, space="PSUM")

        # ---------------- constants / parameters ----------------
        I128b = const.tile([128, 128], BF16, name="I128b")
        make_identity(nc, I128b)
        I128h = pro.tile([128, 128], F16, name="I128h")
        make_identity(nc, I128h)
        dmask = const.tile([128, 128], BF16, name="dmask")
        if self_link == 0:
            nc.gpsimd.memset(dmask, 1.0)
            nc.gpsimd.affine_select(
                out=dmask,
                in_=dmask,
                compare_op=ALU.not_equal,
                fill=0.0,
                base=0,
                pattern=[[-1, 128]],
                channel_multiplier=1,
            )

        wt = pro.tile([128, 2, 128], F32R, name="wt")
        nc.sync.dma_start(wt, WT[:].rearrange("(k p) d -> p k d", p=128))
        wb = const.tile([128, 1], F32, name="wb")
        nc.sync.dma_start(wb, Wb[:])
        acat_f = pro.tile([128, 2], F32, name="acat_f")
        nc.sync.dma_start(acat_f, a_cat[:])

        # a_src column (f32r) for the s_src matmuls
        asrc_r = pro.tile([128, 1], F32R, name="asrc_r")
        nc.vector.tensor_copy(asrc_r, acat_f[:, 0:1])
        # a_dst replicated across 128 columns (weights for the fused
        # s_dst-broadcast matmul): lhsT[d, m] = a_dst[d] for all m.
        adst_rep = pro.tile([128, 128], F32R, name="adst_rep")
        nc.vector.tensor_copy(adst_rep, acat_f[:, 1:2].to_broadcast((128, 128)))

        ones11 = pro.tile([1, 1], F32, name="ones11")
        nc.vector.memset(ones11, 1.0)
        ab_col = const.tile([128, 1], F32, name="ab_col")
        nc.vector.memset(ab_col, float(a_b))

        # ---------------- hT groups + fused sdst_rep ------------------
        # Per 2048-col group: DMA doc.T quarter, matmul hT, ACT-copy to
        # f32r + bf16, fused broadcast matmul -> sdst_rep (+a_b).
        hT_g = [pro.tile([128, GRP], F32R, name=f"hT{g}") for g in range(4)]
        hTb_g = [pro.tile([128, GRP], BF16, name=f"hTb{g}") for g in range(4)]
        sdst_rep = [
            const.tile([128, HALF], F16, name=f"sdst{h}") for h in range(2)
        ]
        s_src_row = pro.tile([1, R], F32, name="s_src_row")

        for g in range(4):
            dT = pro.tile([128, 2, GRP], F32R, tag="dT", bufs=2)
            nc.sync.dma_start(
                dT,
                docT[:, ds(g * GRP, GRP)].rearrange("(k p) n -> p k n", p=128),
            )
            for w in range(4):
                off = g * GRP + w * 512
                ph = ppro.tile([128, 512], F32, tag="pp")
                for k in range(2):
                    nc.tensor.matmul(
                        ph,
                        wt[:, k, :],
                        dT[:, k, ds(w * 512, 512)],
                        start=(k == 0),
                        stop=(k == 1),
                    )
                nc.scalar.activation(
                    hT_g[g][:, ds(w * 512, 512)], ph, AF.Identity, bias=wb
                )
                nc.scalar.activation(
                    hTb_g[g][:, ds(w * 512, 512)], ph, AF.Identity, bias=wb
                )
                # fused s_dst broadcast: [128,512] = adst_rep.T @ hT window
                pr = ppro.tile([128, 512], F32, tag="pp")
                nc.tensor.matmul(
                    pr, adst_rep, hT_g[g][:, ds(w * 512, 512)]
                )
                nc.vector.tensor_scalar_add(
                    sdst_rep[off // HALF][:, ds(off % HALF, 512)],
                    pr,
                    float(a_b),
                )
                # s_src for local rows (first 1024 cols only)
                if off < R:
                    psx = ppro.tile([1, 512], F32, tag="pp")
                    nc.tensor.matmul(
                        psx, asrc_r, hT_g[g][:, ds(w * 512, 512)]
                    )
                    nc.vector.tensor_copy(s_src_row[:, ds(off, 512)], psx)

        # ------------- per-chunk bias columns [128, 8] ----------------
        # scol_dst_ab[:, c] = s_dst[c*128 + p] + a_b  (transpose of the
        # replicated sdst_rep block); scol_src[:, c] = s_src[c*128 + p].
        scol_src = const.tile([128, NCHUNK], F32, name="scol_src")
        scol_dst_ab = const.tile([128, NCHUNK], F32, name="scol_dst_ab")
        for c in range(NCHUNK):
            pc = ppro.tile([128, 1], F32, tag="pp")
            nc.tensor.matmul(
                pc,
                s_src_row[:, ds(c * 128, 128)],
                ones11,
            )
            nc.vector.tensor_copy(scol_src[:, c : c + 1], pc)
            pt = ppro.tile([128, 128], F16, tag="ppb")
            nc.tensor.transpose(
                pt, sdst_rep[0][:, ds(c * 128, 128)], I128h
            )
            nc.vector.tensor_copy(scol_dst_ab[:, c : c + 1], pt[:, 0:1])

        scol_src01 = const.tile([128, NCHUNK], F32, name="scol_src01")
        nc.vector.tensor_scalar_mul(scol_src01, scol_src, SLOPE)

        # E_ii for all chunks at once: exp(lrelu(s_src + s_dst + a_b))
        eii = const.tile([128, NCHUNK], F32, name="eii")
        if self_link == 0:
            nc.vector.tensor_tensor(eii, scol_dst_ab, scol_src, ALU.add)
            nc.scalar.activation(eii, eii, AF.Prelu, alpha=SLOPE)
            nc.scalar.activation(eii, eii, AF.Exp)

        # ------------- h in [j, d] bf16 layout ------------------------
        h_bf = const.tile([128, 64, 128], BF16, name="h_bf")
        for w in range(8):
            phb = ppro.tile([128, 8, 128], BF16, tag="ppb")
            for q in range(8):
                t_ = 8 * w + q
                g, r_ = divmod(t_ * 128, GRP)
                nc.tensor.transpose(
                    phb[:, q], hTb_g[g][:, ds(r_, 128)], I128b
                )
            nc.vector.tensor_copy(h_bf[:, ds(8 * w, 8), :], phb)

        ppro.release()
        pro.release()

        work = tc.alloc_tile_pool(name="work", bufs=2)
        attp = tc.alloc_tile_pool(name="attp", bufs=2)
        ptp = tc.alloc_tile_pool(name="ptp", bufs=3, space="PSUM")
        pho = tc.alloc_tile_pool(name="pho", bufs=2, space="PSUM")

        att_eng = nc.gpsimd if ATT_ENGINE == "gpsimd" else nc.vector

        # =================== main loop over 8 row chunks ==============
        for c in [c for _ in range(reps) for c in range(NCHUNK)]:
            sacc = small.tile([128, 2], F32, tag="sacc")
            nc.vector.memset(sacc, 0.0)
            E = work.tile([128, N], BF16, tag="E", bufs=3)
            for h in range(2):
                L = work.tile([128, HALF], F16, tag="L", bufs=3)
                if 2 * c + h < N_DVE_HALVES:
                    # leaky-relu on DVE in fp16: max(x, 0.1x)
                    sc = work.tile([128, HALF], F16, tag="sc")
                    nc.vector.tensor_scalar_add(
                        sc, sdst_rep[h], scol_src[:, c : c + 1]
                    )
                    sc01 = work.tile([128, HALF], F16, tag="sc01")
                    nc.vector.tensor_scalar(
                        sc01,
                        sdst_rep[h],
                        SLOPE,
                        scol_src01[:, c : c + 1],
                        ALU.mult,
                        ALU.add,
                    )
                    nc.vector.tensor_tensor(L, sc, sc01, ALU.max)
                else:
                    nc.scalar.activation(
                        L,
                        sdst_rep[h],
                        AF.Prelu,
                        bias=scol_src[:, c : c + 1],
                        alpha=SLOPE,
                    )
                nc.scalar.activation(
                    E[:, ds(h * HALF, HALF)],
                    L,
                    AF.Exp,
                    accum_out=sacc[:, h : h + 1],
                )

            s_c = small.tile([128, 1], F32, tag="s_c")
            nc.vector.tensor_tensor(s_c, sacc[:, 0:1], sacc[:, 1:2], ALU.add)
            if self_link == 0:
                # zero the diagonal block; subtract its exp from rowsum
                nc.vector.tensor_tensor(
                    E[:, ds(c * 128, 128)],
                    E[:, ds(c * 128, 128)],
                    dmask,
                    ALU.mult,
                )
                nc.vector.tensor_tensor(
                    s_c, s_c, eii[:, c : c + 1], ALU.subtract
                )

            sinv = small.tile([128, 1], F32, tag="sinv")
            nc.vector.reciprocal(sinv, s_c)

            # att rows out (2MB stores for HBM efficiency)
            for q in range(2):
                at = attp.tile([128, 4096], F32, tag="at")
                att_eng.tensor_scalar_mul(at, E[:, ds(q * 4096, 4096)], sinv)
                nc.sync.dma_start(
                    att_blk[ds(c * 128, 128), ds(q * 4096, 4096)], at
                )

            # E_T tiles + matmul
            ET = work.tile([128, 64, 128], BF16, tag="ET")
            for t in range(4):
                pt = ptp.tile([128, 16, 128], BF16, tag="pt")
                for q in range(16):
                    nc.tensor.transpose(
                        pt[:, q], E[:, ds((16 * t + q) * 128, 128)], I128b
                    )
                nc.vector.tensor_copy(ET[:, ds(16 * t, 16), :], pt)

            ph_o = pho.tile([128, 128], F32, tag="ph_o")
            for jt in range(64):
                nc.tensor.matmul(
                    ph_o,
                    ET[:, jt, :],
                    h_bf[:, jt, :],
                    start=(jt == 0),
                    stop=(jt == 63),
                )
            ho = small.tile([128, 128], F32, tag="ho")
            nc.scalar.activation(ho, ph_o, AF.Prelu, scale=sinv, alpha=SLOPE)
            nc.sync.dma_start(hout_blk[ds(c * 128, 128), :], ho)

        pho.release()
        ptp.release()
        attp.release()
        work.release()
        small.release()
        const.release()

    nc.compile()
    return nc


def kernel(doc, W_w, W_b, a_w, a_b, selfLink):
    global LAST_RESULTS
    doc = np.ascontiguousarray(np.asarray(doc, dtype=np.float32))
    W_w = np.asarray(W_w, dtype=np.float32)
    W_b = np.asarray(W_b, dtype=np.float32)
    a_w = np.asarray(a_w, dtype=np.float32)
    a_b_f = float(np.asarray(a_b, dtype=np.float32).reshape(-1)[0])
    sl = np.asarray(selfLink)
    self_link = int(sl.reshape(-1)[0]) if sl.size else 0

    nc = _build(a_b_f, self_link)

    WT = np.ascontiguousarray(W_w.T)                      # [256, 128]
    Wbv = np.ascontiguousarray(W_b.reshape(D, 1))         # [128, 1]
    acat = np.ascontiguousarray(
        np.stack([a_w[0, :D], a_w[0, D:]], axis=1)        # [128, 2]
    )

    in_maps = []
    for c in range(NCORES):
        docR = np.roll(doc, -c * R, axis=0)
        in_maps.append(
            {
                "docT": np.ascontiguousarray(docR.T),
                "WT": WT,
                "Wb": Wbv,
                "a_cat": acat,
            }
        )

    LAST_RESULTS = run_bass_kernel_spmd(
        nc, in_maps, core_ids=list(range(NCORES))
    )
    results = LAST_RESULTS.results

    att = np.empty((N, N), dtype=np.float32)
    hout = np.empty((N, D), dtype=np.float32)
    for c in range(NCORES):
        att[c * R : (c + 1) * R] = np.roll(results[c]["att_blk"], c * R, axis=1)
        hout[c * R : (c + 1) * R] = results[c]["hout_blk"]
    return (hout, att)
